# revision 48
# baseline (speedup 1.0000x reference)
"""GNN message-passing (PNA-style) Trainium2 Bass kernel, 8-core SPMD.

Self-contained: hardcodes problem shapes. kernel(**inputs) -> [4000, 1] f32.

Design: fp16 node features in 192B rows (super4 768B gather elements,
idx = row>>2 int16); chunk-wide DVE lane-select with pair-duplicated masks
so every elementwise op runs in DVE 2x mode; fp16 fold trees for
min/max/sum/sumsq (squares pre-scaled by 1/64); per-part h_dst staging;
split AllGather per part for compute/collective overlap, with the next
part's first gather prefetched into a static buffer.
"""
import sys
sys.path.insert(0, "/opt/trn_rl_repo")
import numpy as np

import concourse.bass as bass
import concourse.bacc as bacc
import concourse.tile as tile
from concourse import mybir
from concourse.bass_utils import run_bass_kernel_spmd
from concourse.masks import make_identity

fp32 = mybir.dt.float32
fp16 = mybir.dt.float16
i16 = mybir.dt.int16
AF = mybir.ActivationFunctionType
OP = mybir.AluOpType

# problem constants
N, E, G, D = 100000, 400000, 4000, 70
NC = 8
EW = 96             # padded row width (elements) = 192 B (x4 rows = 768 B elems)
EWP = 128           # pooling-table row width (single-row 256 B gather)
ATOM_DIMS = np.array([119, 5, 12, 12, 10, 6, 6, 2, 2])
ATOM_OFFSETS = np.concatenate([[0], np.cumsum(ATOM_DIMS)[:-1]]).astype(np.int64)
DEG_HIST = np.array([0.0, 100.0, 400.0, 300.0, 200.0])
_bins = np.arange(len(DEG_HIST), dtype=np.float64)
AVG_LOG = float((np.log(_bins + 1.0) * DEG_HIST).sum() / DEG_HIST.sum())
BN_EPS = 1e-5
STD_EPS = 1e-5
P = 128
EA_PAD = -1000.0    # pad-slot ea value (message -> 0 after relu; fp16-safe)
BIG = 1000.0        # min-mask additive for tail padding
MAXCOLS = 32        # max gather-chunk columns (nb*d)
NBD_CAP = 32        # max nb*d per block
NB_MAX = 12         # max tiles per block
SQ_SCALE = 0.125    # square pre-scale; s2 accumulates m^2/64 in fp16
SPLITS = (0.60,)             # column fractions per part; last part is the
NPARTS = len(SPLITS) + 1     # remainder (small -> small boundary allgather)


def _insert_axis(ap_obj, pos, count):
    lst = [list(x) for x in ap_obj.ap]
    lst = lst[:pos] + [[0, count]] + lst[pos:]
    return bass.AP(ap_obj.tensor, ap_obj.offset, lst)


def _pairs(ap_obj):
    """Split the (unit-stride, even) last axis into (n/2, 2) so DVE 2x mode
    engages even when another operand broadcasts over the halves."""
    lst = [list(x) for x in ap_obj.ap]
    st, n = lst[-1]
    assert st == 1 and n % 2 == 0, (st, n)
    lst = lst[:-1] + [[2, n // 2], [1, 2]]
    return bass.AP(ap_obj.tensor, ap_obj.offset, lst)


def _fold(nc, spool, g4, d, nb, out_slice, op, dt, tag):
    """Tree-reduce [P, nb, d, 70] over axis j into out_slice [P, nb, 70].

    Contiguous inner-f access on every level. op=add avoids double counting;
    min/max use overlapping halves.
    """
    OPa = mybir.AluOpType
    is_add = op == OPa.add
    k = d
    cur = g4
    first = True
    while k > 1:
        if k == 2:
            nc.vector.tensor_tensor(
                out=out_slice,
                in0=cur[:, :, 0:1].rearrange("p t j f -> p t (j f)"),
                in1=cur[:, :, 1:2].rearrange("p t j f -> p t (j f)"), op=op)
            return
        h = k // 2 if is_add else (k + 1) // 2
        rem = (k - h) if is_add else h
        if first:
            scr = spool.tile([128, nb * rem * 70], dt, tag=tag)
            scr3 = scr[:].rearrange("p (t j f) -> p t j f", t=nb, j=rem)
            nc.vector.tensor_tensor(out=scr3[:, :, 0:h], in0=cur[:, :, 0:h],
                                    in1=cur[:, :, k - h:k], op=op)
            if is_add and k % 2 == 1:
                nc.vector.tensor_copy(out=scr3[:, :, h:h + 1], in_=cur[:, :, h:h + 1])
            cur = scr3
            first = False
        else:
            nc.vector.tensor_tensor(out=cur[:, :, 0:h], in0=cur[:, :, 0:h],
                                    in1=cur[:, :, k - h:k], op=op)
        k = rem


def _wrap16(flat):
    """int16 slot array -> [128, ceil(n/16)] wrapped (i -> (i%16, i//16)), x8 replicated."""
    n = len(flat)
    n16 = (n + 15) // 16
    a = np.zeros(n16 * 16, np.int16)
    a[:n] = flat
    w = a.reshape(n16, 16).T  # [16, n16]
    return np.tile(w, (8, 1)).copy()  # [128, n16]


def _prep(x, edge_index, edge_attr, batch, atom_emb):
    src = np.asarray(edge_index[0], np.int64)
    dst = np.asarray(edge_index[1], np.int64)
    batch = np.asarray(batch, np.int64)
    ea = np.asarray(edge_attr, np.float32)

    deg = np.bincount(dst, minlength=N)
    eorder = np.argsort(dst, kind="stable")
    rowptr = np.zeros(N + 1, np.int64)
    rowptr[1:] = np.cumsum(deg)

    # graph-aligned core node ranges
    gcnt = np.bincount(batch, minlength=G)
    gnode_start = np.zeros(G + 1, np.int64)
    gnode_start[1:] = np.cumsum(gcnt)
    core_gb = [0]
    for c in range(1, NC):
        target = c * N // NC
        gi = int(np.searchsorted(gnode_start, target))
        if gnode_start[gi] != target and gi > 0:
            gi = gi if abs(gnode_start[gi] - target) < abs(gnode_start[gi - 1] - target) else gi - 1
        core_gb.append(gi)
    core_gb.append(G)
    core_nodes = [(int(gnode_start[core_gb[c]]), int(gnode_start[core_gb[c + 1]])) for c in range(NC)]

    dmax = int(deg.max())
    exact_ds = list(range(0, min(dmax, 8) + 1))
    has_tail = dmax > 8
    dtail = dmax if has_tail else 0

    core_group_nodes = []
    for c in range(NC):
        n0, n1 = core_nodes[c]
        nd = deg[n0:n1]
        groups = [np.nonzero(nd == d)[0] + n0 for d in exact_ds]
        if has_tail:
            groups.append(np.nonzero(nd >= 9)[0] + n0)
        core_group_nodes.append(groups)

    ngroups = len(exact_ds) + (1 if has_tail else 0)
    dvals = exact_ds + ([dtail] if has_tail else [])
    NT_g = [max((len(core_group_nodes[c][g]) + P - 1) // P for c in range(NC)) for g in range(ngroups)]
    NT = 1 + sum(NT_g)          # +1 reserved front zero tile
    NB = NT * P
    NPAD = NC * NB
    assert NPAD % 4 == 0 and NPAD // 4 <= 32767, NPAD

    # proc order: tile 0 reserved (all pad), then groups
    proc = np.full((NC, NB), -1, np.int64)
    goff = []
    ti = 1
    for g in range(ngroups):
        goff.append(ti)
        ti += NT_g[g]
    for c in range(NC):
        for g in range(ngroups):
            nodes = core_group_nodes[c][g]
            off = goff[g] * P
            proc[c, off:off + len(nodes)] = nodes

    pos_of_node = np.full(N, -1, np.int64)   # local pos within owning core
    core_of_node = np.full(N, -1, np.int64)
    for c in range(NC):
        mask = proc[c] >= 0
        pos_of_node[proc[c][mask]] = np.nonzero(mask)[0]
        core_of_node[proc[c][mask]] = c
    assert (pos_of_node >= 0).sum() == N

    # blocks: per group, tiles chunked, nb*d <= NBD_CAP
    blocks = []  # (g, d, t0, nb)
    for g in range(ngroups):
        d = dvals[g]
        nb_max = NB_MAX if d == 0 else max(1, min(NB_MAX, NBD_CAP // d))
        for b0 in range(0, NT_g[g], nb_max):
            nb = min(nb_max, NT_g[g] - b0)
            blocks.append((g, d, goff[g] + b0, nb))

    # part boundaries at block boundaries (by column count)
    totcols = sum(d * nb for (_, d, _, nb) in blocks if d > 0)
    cumsplit = np.cumsum(SPLITS)
    part_of_block = []
    cum = 0
    for (g, d, t0, nb) in blocks:
        frac = cum / max(totcols, 1)
        part_of_block.append(int(np.searchsorted(cumsplit, frac, side="right")))
        cum += d * nb
    # part boundaries as tile indices: TS[k] = min tile of part k
    TS = [min((b[2] for b, p in zip(blocks, part_of_block) if p >= k), default=NT)
          for k in range(1, NPARTS)]
    # parts must be tile-contiguous: blocks are emitted group-major == tile order
    TSx = [0] + TS + [NT]
    for b, p in zip(blocks, part_of_block):
        t0, nb = b[2], b[3]
        assert TSx[p] <= t0 and t0 + nb <= TSx[p + 1]
    HS = [t * P for t in TSx]  # row boundaries, len NPARTS+1 (HS[0]=0, HS[-1]=NB)

    def g2row(c, pos):
        for k in range(NPARTS):
            if pos < HS[k + 1]:
                return NC * HS[k] + c * (HS[k + 1] - HS[k]) + (pos - HS[k])
        raise AssertionError(pos)

    # column layout + gather metadata
    ea_offsets = []   # (g, d, t0, nb, col, ea_off, bcol)
    col = 0
    ea_off = 0
    bcol = 0
    for (g, d, t0, nb) in blocks:
        if d == 0:
            continue
        ea_offsets.append((g, d, t0, nb, col, ea_off, bcol))
        col += nb * d
        ea_off += P * nb * d * D
        if has_tail and g == ngroups - 1:
            bcol += nb * d
    NIDX = col
    TOTEA = max(ea_off, 1)
    bigpad_cols = max(bcol, 1)

    srcrow = np.zeros((NC, P, max(NIDX, 1)), np.int64)   # g2 row per slot (0 for pad)
    lane_m = np.zeros((NC, 4, P, max(NIDX, 1), 2), np.float16)  # pair-dup for DVE 2x
    eaflat = np.zeros((NC, TOTEA), np.float16)
    bigpad = np.zeros((NC, P, bigpad_cols), np.float16)

    ea16 = ea.astype(np.float16)
    for c in range(NC):
        for (g, d, t0, nb, col0, eo, bc0) in ea_offsets:
            blk = np.full((P, nb * d, D), EA_PAD, np.float16)
            for i in range(nb):
                t = t0 + i
                for p in range(P):
                    node = proc[c, t * P + p]
                    if node < 0:
                        continue
                    nd = int(deg[node])
                    use = min(nd, d)
                    eids = eorder[rowptr[node]:rowptr[node] + use]
                    blk[p, i * d:i * d + use] = ea16[eids]
                    ss = src[eids]
                    rows = np.array([g2row(int(core_of_node[s]), int(pos_of_node[s])) for s in ss], np.int64)
                    srcrow[c, p, col0 + i * d:col0 + i * d + use] = rows
                    lane_m[c, rows & 3, p, col0 + i * d + np.arange(use), :] = 1.0
                    if g == ngroups - 1 and has_tail and nd < d:
                        bigpad[c, p, bc0 + i * d + nd:bc0 + (i + 1) * d] = BIG
            eaflat[c, eo:eo + P * nb * d * D] = blk.reshape(-1)

    # idx16: slot i = col*128 + p -> srcrow >> 2
    idx16 = np.zeros((NC, 128, max(NIDX * 8, 1)), np.int16)
    for c in range(NC):
        flat = (srcrow[c].T.reshape(-1) >> 2).astype(np.int16)  # [NIDX*128] in (col, p) order
        idx16[c, :, :len(flat) // 16] = _wrap16(flat)[:, :len(flat) // 16]

    # statics [NC, 4, NT, P]: 1/deg_safe, amp, 1/amp, 64/deg_safe
    statn = np.zeros((NC, 4, NT, P), np.float32)
    for c in range(NC):
        nodes = proc[c]
        dd = np.where(nodes >= 0, deg[np.clip(nodes, 0, N - 1)], 0).astype(np.float64)
        dsafe = np.maximum(dd, 1.0)
        amp = np.log(dsafe + 1.0) / AVG_LOG
        statn[c, 0] = (1.0 / dsafe).astype(np.float32).reshape(NT, P)
        statn[c, 1] = amp.astype(np.float32).reshape(NT, P)
        statn[c, 2] = (1.0 / amp).astype(np.float32).reshape(NT, P)
        statn[c, 3] = (64.0 / dsafe).astype(np.float32).reshape(NT, P)

    # h0 (AtomEncoder), padded fp16, proc order + g2 full table
    xl = np.asarray(x, np.int64)
    emb = np.asarray(atom_emb, np.float32)
    h0_all = emb[xl + ATOM_OFFSETS[None, :]].sum(axis=1)
    h0_own = np.zeros((NC, NB, EW), np.float16)
    for c in range(NC):
        mask = proc[c] >= 0
        h0_own[c][mask, :D] = h0_all[proc[c][mask]].astype(np.float16)
    h0_full = np.zeros((NPAD, EW), np.float16)
    for c in range(NC):
        for pos0 in range(0, NB, P):
            rows = np.array([g2row(c, pos0 + p) for p in range(P)])
            h0_full[rows] = h0_own[c, pos0:pos0 + P]

    # pooling: per core graphs sorted by size desc; common tile grid
    core_graphs = []
    for c in range(NC):
        gids = np.arange(core_gb[c], core_gb[c + 1])
        order = np.argsort(-gcnt[gids], kind="stable")
        core_graphs.append(gids[order])
    NGT = max((len(cg) + P - 1) // P for cg in core_graphs)
    KG_t = []
    for t in range(NGT):
        m = 1
        for c in range(NC):
            cg = core_graphs[c]
            if t * P < len(cg):
                m = max(m, int(gcnt[cg[t * P]]))
        KG_t.append(m)
    npoolcols = sum(KG_t)
    poolpos = np.zeros((NC, P, npoolcols), np.int64)  # local pos; 0 = reserved zero row
    ginv = np.ones((NC, NGT, P), np.float32)
    pc = 0
    pool_cols = []
    for t in range(NGT):
        pool_cols.append(pc)
        for c in range(NC):
            cg = core_graphs[c]
            for p in range(P):
                if t * P + p >= len(cg):
                    continue
                gid = cg[t * P + p]
                sz = int(gcnt[gid])
                ginv[c, t, p] = 1.0 / max(sz, 1)
                if sz > 0:
                    nids = np.arange(gnode_start[gid], gnode_start[gid] + sz)
                    poolpos[c, p, pc:pc + sz] = pos_of_node[nids]
        pc += KG_t[t]
    poolidx16 = np.zeros((NC, 128, max(npoolcols * 8, 1)), np.int16)
    for c in range(NC):
        flat = poolpos[c].T.reshape(-1).astype(np.int16)
        poolidx16[c, :, :len(flat) // 16] = _wrap16(flat)[:, :len(flat) // 16]

    # gather chunks per part: pack d>0 blocks up to MAXCOLS columns
    chunksP = tuple([] for _ in range(NPARTS))
    d0P = tuple([] for _ in range(NPARTS))
    for bi, b in enumerate(blocks):
        (g, d, t0, nb) = b
        pnum = part_of_block[bi]
        if d == 0:
            d0P[pnum].append(b)
    for pnum in range(NPARTS):
        cur = []
        cc = 0
        for eb in ea_offsets:
            (g, d, t0, nb, col0, eo, bc0) = eb
            bpart = int(np.searchsorted(TS, t0, side="right"))
            if bpart != pnum:
                continue
            if cur and cc + nb * d > MAXCOLS:
                chunksP[pnum].append(cur)
                cur = []
                cc = 0
            cur.append(eb)
            cc += nb * d
        if cur:
            chunksP[pnum].append(cur)

    cfg = dict(NB=NB, NT=NT, NPAD=NPAD, NIDX=max(NIDX, 1), TOTEA=TOTEA,
               HS=HS,
               blocks=blocks, ea_offsets=ea_offsets, dvals=dvals, goff=goff,
               NT_g=NT_g, ngroups=ngroups, has_tail=has_tail, dtail=dtail,
               NGT=NGT, KG_t=KG_t, pool_cols=pool_cols, npoolcols=npoolcols,
               bigpad_cols=bigpad_cols, chunksP=chunksP, d0P=d0P)
    arrays = dict(idx16=idx16, lane_m=lane_m, eaflat=eaflat, statn=statn,
                  h0_own=h0_own, h0_full=h0_full, poolidx16=poolidx16, ginv=ginv,
                  bigpad=bigpad)
    asm = dict(core_graphs=core_graphs, core_gb=core_gb)
    return cfg, arrays, asm


def _prep_weights(post_w, post_b, bn_gamma, bn_beta, mlp_w1, mlp_b1, mlp_w2, mlp_b2, mlp_w3, mlp_b3):
    post_w = np.asarray(post_w, np.float32)
    post_b = np.asarray(post_b, np.float32)
    bn_gamma = np.asarray(bn_gamma, np.float32)
    bn_beta = np.asarray(bn_beta, np.float32)
    inv_std_bn = np.float32(1.0 / np.sqrt(1.0 + BN_EPS))
    wch = np.zeros((4, 3, P, 210), np.float16)
    for l in range(4):
        for ch in range(3):
            r0, r1 = ch * 128, min((ch + 1) * 128, 280)
            rows = r1 - r0
            for s in range(3):
                wch[l, ch, :rows, s * 70:(s + 1) * 70] = post_w[l, s * 280 + r0:s * 280 + r1, :].astype(np.float16)
    Grep = bn_gamma * inv_std_bn
    B2 = post_b * Grep + bn_beta
    w1 = np.asarray(mlp_w1, np.float32)
    w2 = np.asarray(mlp_w2, np.float32)
    w3 = np.asarray(mlp_w3, np.float32)
    reps = np.concatenate([Grep.ravel(), B2.ravel(), np.asarray(mlp_b1, np.float32),
                           np.asarray(mlp_b2, np.float32), np.asarray(mlp_b3, np.float32)]).astype(np.float32)
    reps = np.broadcast_to(reps, (P, reps.size)).copy()
    return dict(wch=wch, reps=reps, w1=w1, w2=w2, w3=w3)


def _build(cfg):
    NB, NT, NPAD, NIDX, TOTEA = cfg["NB"], cfg["NT"], cfg["NPAD"], cfg["NIDX"], cfg["TOTEA"]
    NGT, npoolcols = cfg["NGT"], cfg["npoolcols"]
    NREP = 4 * 70 + 4 * 70 + 35 + 17 + 1
    NI16 = max(NIDX * 8, 1)
    NPI16 = max(npoolcols * 8, 1)

    nc = bacc.Bacc("TRN2", target_bir_lowering=False, debug=False, num_devices=NC,
                   num_swdge_queues=4)
    qrr = [0]  # SWDGE queue round-robin (desc-gen parallelizes across queues)

    def next_q():
        qrr[0] += 1
        return qrr[0] % 4
    h0_own = nc.dram_tensor("h0_own", [NB, EW], fp16, kind="ExternalInput").ap()
    h0_full = nc.dram_tensor("h0_full", [NPAD, EW], fp16, kind="ExternalInput").ap()
    idx16 = nc.dram_tensor("idx16", [128, NI16], i16, kind="ExternalInput").ap()
    lane_m = nc.dram_tensor("lane_m", [4, P, NIDX * 2], fp16, kind="ExternalInput").ap()
    eaflat = nc.dram_tensor("eaflat", [TOTEA], fp16, kind="ExternalInput").ap()
    statn16 = nc.dram_tensor("statn16", [P, 4 * NT * 2], fp16, kind="ExternalInput").ap()
    reps16_t = nc.dram_tensor("reps16", [P, NREP], fp16, kind="ExternalInput").ap()
    bigpad_t = nc.dram_tensor("bigpad", [P, cfg["bigpad_cols"]], fp16, kind="ExternalInput").ap()
    poolidx16 = nc.dram_tensor("poolidx16", [128, NPI16], i16, kind="ExternalInput").ap()
    ginv = nc.dram_tensor("ginv", [NGT, P], fp32, kind="ExternalInput").ap()
    wch = nc.dram_tensor("wch", [4, 3, P, 210], fp16, kind="ExternalInput").ap()
    reps = nc.dram_tensor("reps", [P, NREP], fp32, kind="ExternalInput").ap()
    w1 = nc.dram_tensor("w1", [D, 35], fp32, kind="ExternalInput").ap()
    w2 = nc.dram_tensor("w2", [35, 17], fp32, kind="ExternalInput").ap()
    w3 = nc.dram_tensor("w3", [17, 1], fp32, kind="ExternalInput").ap()
    out_g = nc.dram_tensor("out_g", [NGT * P, 1], fp32, kind="ExternalOutput").ap()

    h_own = [None] + [nc.dram_tensor(f"h_own{l}", [NB, EW if l < 4 else EWP], fp16)
                      for l in range(1, 5)]
    hbuf = [None] + [nc.dram_tensor(f"hbuf{l}", [NPAD, EW], fp16, addr_space="Shared")
                     for l in range(1, 4)]

    # persistent SBUF
    # per-gather-call idx tensors: dma_gather needs idxs_ap at tensor base
    # and num_idxs <= 1024 (HW limit) -> <= GCAP columns per call
    GCAP = 8
    chunk_idx_sb = {}
    for part in range(NPARTS):
        for ci, chunk in enumerate(cfg["chunksP"][part]):
            ncols = sum(nb * d for (_, d, _, nb, _, _, _) in chunk)
            for k in range((ncols + GCAP - 1) // GCAP):
                w = min(GCAP, ncols - k * GCAP)
                chunk_idx_sb[(part, ci, k)] = nc.alloc_sbuf_tensor(
                    f"cidx_{part}_{ci}_{k}", [128, w * 8], i16).ap()
    pool_idx_sb = {}
    for t in range(cfg["NGT"]):
        KG = cfg["KG_t"][t]
        for k in range((KG + GCAP - 1) // GCAP):
            w = min(GCAP, KG - k * GCAP)
            pool_idx_sb[(t, k)] = nc.alloc_sbuf_tensor(
                f"pidx_{t}_{k}", [128, w * 8], i16).ap()
    mask_sb = nc.alloc_sbuf_tensor("mask_sb", [P, 4 * NIDX * 2], fp16).ap()
    statn16_sb = nc.alloc_sbuf_tensor("statn16_sb", [P, 4 * NT * 2], fp16).ap()
    reps16_sb = nc.alloc_sbuf_tensor("reps16_sb", [P, NREP], fp16).ap()
    wch_sb = nc.alloc_sbuf_tensor("wch_sb", [P, 4 * 3 * 210], fp16).ap()
    reps_sb = nc.alloc_sbuf_tensor("reps_sb", [P, NREP], fp32).ap()
    w1_sb = nc.alloc_sbuf_tensor("w1_sb", [D, 35], fp32).ap()
    w2_sb = nc.alloc_sbuf_tensor("w2_sb", [35, 17], fp32).ap()
    w3_sb = nc.alloc_sbuf_tensor("w3_sb", [17, 1], fp32).ap()
    ident16 = nc.alloc_sbuf_tensor("ident16", [P, P], fp16).ap()
    ident32 = nc.alloc_sbuf_tensor("ident32", [P, P], fp32).ap()
    epsb = nc.alloc_sbuf_tensor("epsb", [P, 1], fp32).ap()
    zrow = nc.alloc_sbuf_tensor("zrow", [P, EWP], fp16).ap()

    cc_sems = {(l, h): nc.alloc_semaphore(name=f"ccs{l}_{h}")
               for l in range(1, 4) for h in range(NPARTS)}

    # static gather buffers for each later part's first chunk: filled at the
    # end of the previous part's context so the next part starts computing
    # immediately (context exit drains, so cross-context R/W order is safe)
    pf_gt = {}

    HS = cfg["HS"]
    part_rows = [(HS[k], HS[k + 1]) for k in range(NPARTS)]
    part_out = [(NC * HS[k], NC * HS[k + 1]) for k in range(NPARTS)]

    # ---- segment 0 ----
    with tile.TileContext(nc) as tc:
        with tc.tile_pool(name="s0", bufs=2) as pool:
            for part in range(NPARTS):
                for ci, chunk in enumerate(cfg["chunksP"][part]):
                    col0 = chunk[0][4]
                    ncols = sum(nb * d for (_, d, _, nb, _, _, _) in chunk)
                    for k in range((ncols + GCAP - 1) // GCAP):
                        w = min(GCAP, ncols - k * GCAP)
                        c0 = col0 + k * GCAP
                        nc.sync.dma_start(out=chunk_idx_sb[(part, ci, k)][:, :],
                                          in_=idx16[:, c0 * 8:(c0 + w) * 8])
            for t in range(cfg["NGT"]):
                pc = cfg["pool_cols"][t]
                KG = cfg["KG_t"][t]
                for k in range((KG + GCAP - 1) // GCAP):
                    w = min(GCAP, KG - k * GCAP)
                    c0 = pc + k * GCAP
                    nc.sync.dma_start(out=pool_idx_sb[(t, k)][:, :],
                                      in_=poolidx16[:, c0 * 8:(c0 + w) * 8])
            nc.sync.dma_start(out=mask_sb[:].rearrange("p (j c) -> p j c", j=4),
                              in_=lane_m.rearrange("j p c -> p j c"))
            nc.sync.dma_start(out=statn16_sb[:, :], in_=statn16[:, :])
            nc.sync.dma_start(out=reps16_sb[:, :], in_=reps16_t[:, :])
            nc.sync.dma_start(out=wch_sb[:].rearrange("p (l c f) -> p l c f", l=4, c=3),
                              in_=wch.rearrange("l c p f -> p l c f"))
            nc.sync.dma_start(out=reps_sb[:, :], in_=reps[:, :])
            nc.sync.dma_start(out=w1_sb[:, :], in_=w1[:, :])
            nc.sync.dma_start(out=w2_sb[:, :], in_=w2[:, :])
            nc.sync.dma_start(out=w3_sb[:, :], in_=w3[:, :])
            make_identity(nc, ident16[:])
            make_identity(nc, ident32[:])
            nc.vector.memset(epsb[:], STD_EPS)
            nc.vector.memset(zrow[:], 0.0)
            # reserved front tile must be finite-zero in every h buffer:
            # it is allgathered and its rows sit inside gathered super4 rows
            for l in range(1, 5):
                w_l = EW if l < 4 else EWP
                nc.sync.dma_start(out=h_own[l].ap()[0:P, :], in_=zrow[:, 0:w_l])

    def do_cc(l, part):
        a, b = part_rows[part]
        oa, ob = part_out[part]
        nc.gpsimd.collective_compute(
            "AllGather", OP.bypass,
            replica_groups=[list(range(NC))],
            ins=[h_own[l].ap()[a:b, :].opt()],
            outs=[hbuf[l].ap()[oa:ob, :].opt()],
        ).then_inc(cc_sems[(l, part)])

    def emit_gather_calls(chunk, hfull_t, part, ci, gt_ap):
        ncols = sum(nb * d for (_, d, _, nb, _, _, _) in chunk)
        for k in range((ncols + GCAP - 1) // GCAP):
            w = min(GCAP, ncols - k * GCAP)
            nc.gpsimd.dma_gather(
                out_ap=gt_ap[:, k * GCAP * 4 * EW:(k * GCAP + w) * 4 * EW].rearrange(
                    "p (c e) -> p c e", e=4 * EW),
                in_ap=bass.AP(hfull_t, 0, [[4 * EW, NPAD // 4], [1, 4 * EW]]),
                idxs_ap=chunk_idx_sb[(part, ci, k)][:, :],
                num_idxs=w * P,
                num_idxs_reg=w * P,
                elem_size=4 * EW,
                queue_num=next_q(),
            )

    def emit_chunk(chunk, l, hfull_t, pools, part, ci, hdstbig, pt0, pre_gt=None):
        """Gather a chunk (<=MAXCOLS cols), chunk-wide select, per-block aggs."""
        (pool, gpool, spool, psp) = pools
        import os as _os
        col0 = chunk[0][4]
        ncols = sum(nb * d for (_, d, _, nb, _, _, _) in chunk)
        XC = ncols * D
        if pre_gt is not None:
            gt = pre_gt  # gathered in the previous part's context
        else:
            gt = gpool.tile([P, ncols * 4 * EW], fp16, tag="gt")
            emit_gather_calls(chunk, hfull_t, part, ci, gt)
        if _os.environ.get("KERNEL_NOCOMPUTE", "0") == "1":
            return
        m = pool.tile([P, XC], fp16, tag="m")
        tsel = pool.tile([P, XC], fp16, tag="tsel")
        eat = pool.tile([P, XC], fp16, tag="eat")
        # per-block ea loads (eaflat rows are [P, X] per block, not chunk-major)
        for (g, d, t0, nb, bcol0, eo, bc) in chunk:
            off = (bcol0 - col0) * D
            X_b = nb * d * D
            nc.sync.dma_start(out=eat[:, off:off + X_b],
                              in_=eaflat[eo:eo + P * X_b].rearrange("(p x) -> p x", p=P))
        # fold h_dst into ea (per block: broadcast shape depends on d)
        for (g, d, t0, nb, bcol0, eo, bc) in chunk:
            off = (bcol0 - col0) * D
            eat4 = eat[:, off:off + nb * d * D].rearrange("p (t j f) -> p t j f", t=nb, j=d)
            hb = _insert_axis(
                hdstbig[:, (t0 - pt0) * D:(t0 - pt0 + nb) * D].rearrange("p (t f) -> p t f", t=nb), 2, d)
            nc.vector.tensor_tensor(out=eat4, in0=eat4, in1=hb, op=OP.add)
        # chunk-wide lane select; pair-split views keep every operand
        # innermost-contiguous (DVE 2x mode needs that on all operands)
        gv = gt[:, 0:ncols * 4 * EW].rearrange("p (c l h r) -> p c l h r", l=4, h=EW // 2, r=2)
        m4 = m[:].rearrange("p (c h r) -> p c h r", h=35, r=2)
        t4 = tsel[:].rearrange("p (c h r) -> p c h r", h=35, r=2)
        mv = mask_sb[:].rearrange("p (j c r) -> p j c r", j=4, r=2)
        for j in range(4):
            mj = _insert_axis(mv[:, j, col0:col0 + ncols], 2, 35)
            dstv = m4 if j == 0 else t4
            nc.vector.tensor_tensor(out=dstv, in0=gv[:, :, j, 0:35], in1=mj, op=OP.mult)
            if j > 0:
                nc.vector.tensor_tensor(out=m[:], in0=m[:], in1=tsel[:], op=OP.add)
        nc.vector.tensor_tensor(out=m[:], in0=m[:], in1=eat[:], op=OP.add)
        nc.scalar.activation(out=m[:], in_=m[:], func=AF.Relu)
        msq = None
        if any(d >= 2 for (_, d, _, _, _, _, _) in chunk):
            msq = pool.tile([P, XC], fp16, tag="msq")
            # pre-scale by 1/8 so sum of squares stays in fp16 range; stage2
            # multiplies by 64/deg to undo
            nc.scalar.activation(out=msq[:], in_=m[:], func=AF.Square, scale=SQ_SCALE)
        for eb in chunk:
            emit_block_aggs(eb, l, m, msq, col0, pool, spool, psp, hdstbig, pt0)

    def emit_block_aggs(eb, l, m_ch, msq_ch, chunk_col0, pool, spool, psp, hdstbig, pt0):
        (g, d, t0, nb, col0, ea_off, bcol) = eb
        X = nb * d * D
        nbd = nb * d
        off = (col0 - chunk_col0) * D
        hdst_ap = hdstbig[:, (t0 - pt0) * D:(t0 - pt0 + nb) * D]

        agg = spool.tile([P, nb * 280], fp16, tag="agg")
        a3 = agg[:].rearrange("p (t f) -> p t f", t=nb)
        is_tail = cfg["has_tail"] and g == cfg["ngroups"] - 1
        mt = m_ch[:, off:off + X].rearrange("p (t j f) -> p t j f", t=nb, j=d)
        if d == 1:
            # mean = min = max = m exactly (deg 1); std handled in stage2
            m3 = m_ch[:, off:off + X].rearrange("p (t f) -> p t f", t=nb)
            nc.vector.tensor_copy(out=a3[:, :, 0:70], in_=m3)
            nc.vector.tensor_copy(out=a3[:, :, 70:140], in_=m3)
            nc.vector.tensor_copy(out=a3[:, :, 140:210], in_=m3)
            s32 = None
            s2 = None
        else:
            # min (tail: masked), max via contiguous fold trees
            if is_tail:
                bp = pool.tile([P, nbd], fp16, tag="bp")
                nc.sync.dma_start(out=bp[:], in_=bigpad_t[:, bcol:bcol + nbd])
                mm = pool.tile([P, X], fp16, tag="tsel")
                mm4 = mm[:].rearrange("p (t j f) -> p t j f", t=nb, j=d)
                bp_b = _insert_axis(bp[:].rearrange("p (t j) -> p t j", t=nb), 3, D)
                nc.vector.tensor_tensor(out=mm4, in0=mt, in1=bp_b, op=OP.add)
                _fold(nc, spool, mm4, d, nb, a3[:, :, 70:140], OP.min, fp16, "fscr")
            else:
                _fold(nc, spool, mt, d, nb, a3[:, :, 70:140], OP.min, fp16, "fscr")
            _fold(nc, spool, mt, d, nb, a3[:, :, 140:210], OP.max, fp16, "fscr")
            s32 = spool.tile([P, nb * D], fp16, tag="s32")
            _fold(nc, spool, mt, d, nb, s32[:].rearrange("p (t f) -> p t f", t=nb),
                  OP.add, fp16, "fscr")
            s2 = spool.tile([P, nb * D], fp16, tag="s2")
            _fold(nc, spool, msq_ch[:, off:off + X].rearrange("p (t j f) -> p t j f", t=nb, j=d),
                  d, nb, s2[:].rearrange("p (t f) -> p t f", t=nb), OP.add, fp16, "fscr")
        _stage2(nc, pool, spool, psp, cfg, None, a3, s32, s2,
                t0, nb, l, wch_sb, reps_sb, ident16, epsb, hdst_ap, h_own[l].ap(), d,
                statn16_sb, reps16_sb)

    def emit_d0_block(blk, l, pool, spool, psp, hdstbig, pt0):
        (g, d, t0, nb) = blk
        hdst_ap = hdstbig[:, (t0 - pt0) * D:(t0 - pt0 + nb) * D]
        agg = spool.tile([P, nb * 280], fp16, tag="agg")
        nc.vector.memset(agg[:], 0.0)
        a3 = agg[:].rearrange("p (t f) -> p t f", t=nb)
        _stage2(nc, pool, spool, psp, cfg, None, a3, None, None,
                t0, nb, l, wch_sb, reps_sb, ident16, epsb, hdst_ap, h_own[l].ap(), d,
                statn16_sb, reps16_sb)

    # ---- layers ----
    import os as _os
    MAXL = int(_os.environ.get("KERNEL_MAXL", "4"))
    NOCC = _os.environ.get("KERNEL_NOCC", "0") == "1"
    for l in range(1, MAXL + 1):
        hfull_t = h0_full.tensor if l == 1 else hbuf[l - 1]
        if l >= 2 and not NOCC:
            for part in range(NPARTS):
                nc.gpsimd.wait_ge(cc_sems[(l - 1, part)], 1)
        for part in range(NPARTS):
            pt0, pt1 = HS[part] // P, HS[part + 1] // P
            hprev_own = h0_own if l == 1 else h_own[l - 1].ap()
            with tile.TileContext(nc) as tc:
                with tc.tile_pool(name=f"L{l}p{part}", bufs=2) as pool, \
                     tc.tile_pool(name=f"Lg{l}p{part}", bufs=3) as gpool, \
                     tc.tile_pool(name=f"Ls{l}p{part}", bufs=2) as spool, \
                     tc.tile_pool(name=f"Lh{l}p{part}", bufs=1) as hpool, \
                     tc.tile_pool(name=f"Lp{l}p{part}", bufs=2, space="PSUM") as psp:
                    pools = (pool, gpool, spool, psp)
                    hdstbig = hpool.tile([P, (pt1 - pt0) * D], fp16, tag="hdstbig")
                    nc.sync.dma_start(
                        out=hdstbig[:],
                        in_=hprev_own[pt0 * P:pt1 * P, 0:D].rearrange("(t p) f -> p t f", p=P))
                    for ci, chunk in enumerate(cfg["chunksP"][part]):
                        emit_chunk(chunk, l, hfull_t, pools, part, ci, hdstbig, pt0)
                    for blk in cfg["d0P"][part]:
                        emit_d0_block(blk, l, pool, spool, psp, hdstbig, pt0)
                    if l == 4 and part == NPARTS - 1:
                        _pooling(nc, pool, spool, psp, cfg, pool_idx_sb, ginv, h_own[4],
                                 w1_sb, w2_sb, w3_sb, reps_sb, ident32, out_g)
            if l < 4 and not NOCC:
                do_cc(l, part)

    nc.compile()
    return nc


def _stage2(nc, pool, spool, psp, cfg, statn_sb, a3, s32, s2,
            t0, nb, l, wch_sb, reps_sb, ident16, epsb, hdst, hout, d,
            statn16_sb, reps16_sb):
    NT = cfg["NT"]

    s16v = statn16_sb[:].rearrange("p (k t r) -> p k t r", k=4, r=2)
    invc16_pb = _insert_axis(s16v[:, 0, t0:t0 + nb], 2, 35)
    amp16_pb = _insert_axis(s16v[:, 1, t0:t0 + nb], 2, 35)
    iamp16_pb = _insert_axis(s16v[:, 2, t0:t0 + nb], 2, 35)
    invc64_16_pb = _insert_axis(s16v[:, 3, t0:t0 + nb], 2, 35)

    if d == 0:
        # agg all zero; std = sqrt(eps)
        nc.scalar.activation(out=a3[:, :, 210:280], in_=a3[:, :, 0:70], func=AF.Sqrt, bias=epsb[:])
    elif d == 1:
        # mean=min=max=m (copied by caller); var = 0 exactly -> std const
        nc.vector.memset(a3[:, :, 210:280], float(np.sqrt(STD_EPS)))
    else:
        s3 = s32[:].rearrange("p (t f) -> p t f", t=nb)
        s23 = s2[:].rearrange("p (t f) -> p t f", t=nb)
        nc.vector.tensor_tensor(out=_pairs(a3[:, :, 0:70]), in0=_pairs(s3),
                                in1=invc16_pb, op=OP.mult)
        u = spool.tile([P, nb * 70], fp16, tag="uv")
        u3 = u[:].rearrange("p (t f) -> p t f", t=nb)
        nc.vector.tensor_tensor(out=_pairs(u3), in0=_pairs(s23),
                                in1=invc64_16_pb, op=OP.mult)
        v = spool.tile([P, nb * 70], fp16, tag="uv")
        v3 = v[:].rearrange("p (t f) -> p t f", t=nb)
        nc.vector.tensor_tensor(out=v3, in0=a3[:, :, 0:70], in1=a3[:, :, 0:70], op=OP.mult)
        nc.vector.tensor_tensor(out=u[:], in0=u[:], in1=v[:], op=OP.subtract)
        nc.scalar.activation(out=u[:], in_=u[:], func=AF.Relu)
        nc.scalar.activation(out=a3[:, :, 210:280], in_=u3, func=AF.Sqrt, bias=epsb[:])

    # scaled copies: agg*amp at 280.., agg/amp handled via weights? no - baseline folds
    # post matmul per tile: psmm[:, 0:70]=A, 70:140=B(amp), 140:210=C(iamp)
    sabc = spool.tile([P, nb * 210], fp16, tag="sabc")
    for i in range(nb):
        aggT = pool.tile([P, 3 * P], fp16, tag="aggT")
        psmm = psp.tile([P, 210], fp32, space="PSUM", tag="psmm")
        psT = psp.tile([P, 3 * P], fp16, space="PSUM", tag="psTx")
        for ch in range(3):
            rows = 128 if ch < 2 else 24
            nc.tensor.transpose(out=psT[:rows, ch * P:(ch + 1) * P],
                                in_=a3[:, i:i + 1, ch * 128:ch * 128 + rows].rearrange("p t f -> p (t f)"),
                                identity=ident16[:])
        # single PSUM->SBUF move for all three transposed chunks
        nc.scalar.activation(out=aggT[:], in_=psT[:], func=AF.Copy)
        for ch in range(3):
            rows = 128 if ch < 2 else 24
            nc.tensor.matmul(out=psmm[:, :], lhsT=aggT[:rows, ch * P:(ch + 1) * P],
                             rhs=wch_sb[:].rearrange("p (l c f) -> p l c f", l=4, c=3)[:rows, l - 1, ch, :],
                             start=(ch == 0), stop=(ch == 2))
        nc.scalar.activation(out=sabc[:, i * 210:(i + 1) * 210], in_=psmm[:, :], func=AF.Copy)

    sA = sabc[:].rearrange("p (t f) -> p t f", t=nb)[:, :, 0:70]
    sB = sabc[:].rearrange("p (t f) -> p t f", t=nb)[:, :, 70:140]
    sC = sabc[:].rearrange("p (t f) -> p t f", t=nb)[:, :, 140:210]
    hn = pool.tile([P, nb * 70], fp16, tag="hn")
    hn3 = hn[:].rearrange("p (t f) -> p t f", t=nb)
    tmp = pool.tile([P, nb * 70], fp16, tag="tmp")
    tmp3 = tmp[:].rearrange("p (t f) -> p t f", t=nb)
    nc.vector.tensor_tensor(out=_pairs(tmp3), in0=_pairs(sB), in1=amp16_pb, op=OP.mult)
    nc.vector.tensor_tensor(out=tmp3, in0=tmp3, in1=sA, op=OP.add)
    tmp2 = pool.tile([P, nb * 70], fp16, tag="tmp2")
    tmp23 = tmp2[:].rearrange("p (t f) -> p t f", t=nb)
    nc.vector.tensor_tensor(out=_pairs(tmp23), in0=_pairs(sC), in1=iamp16_pb, op=OP.mult)
    nc.vector.tensor_tensor(out=tmp3, in0=tmp3, in1=tmp23, op=OP.add)
    # BN affine + relu + residual
    Grep_b = _insert_axis(reps16_sb[:, (l - 1) * 70:l * 70], 1, nb)
    B2_b = _insert_axis(reps16_sb[:, 280 + (l - 1) * 70:280 + l * 70], 1, nb)
    nc.vector.tensor_tensor(out=tmp3, in0=tmp3, in1=Grep_b, op=OP.mult)
    nc.vector.tensor_tensor(out=hn3, in0=tmp3, in1=B2_b, op=OP.add)
    nc.scalar.activation(out=hn[:], in_=hn[:], func=AF.Relu)
    nc.vector.tensor_tensor(out=hn[:], in0=hn[:], in1=hdst, op=OP.add)
    nc.sync.dma_start(out=hout[t0 * P:(t0 + nb) * P, 0:D].rearrange("(t p) f -> p t f", p=P),
                      in_=hn[:].rearrange("p (t f) -> p t f", t=nb))


def _pooling(nc, pool, spool, psp, cfg, pool_idx_sb, ginv, h4, w1_sb, w2_sb, w3_sb, reps_sb, ident32, out_g):
    boff = 560
    GCAP = 8
    for t in range(cfg["NGT"]):
        KG = cfg["KG_t"][t]
        pg = pool.tile([P, KG * 128], fp16, tag="pg")
        for k in range((KG + GCAP - 1) // GCAP):
            w = min(GCAP, KG - k * GCAP)
            nc.gpsimd.dma_gather(
                out_ap=pg[:, k * GCAP * 128:(k * GCAP + w) * 128].rearrange(
                    "p (c e) -> p c e", e=128),
                in_ap=h4.ap()[:, :],
                idxs_ap=pool_idx_sb[(t, k)][:, :],
                num_idxs=w * P,
                num_idxs_reg=w * P,
                elem_size=128,
                queue_num=(t + k) % 4,
            )
        pgv = pg[:].rearrange("p (c e) -> p c e", e=128)
        gsum = pool.tile([P, D], fp32, tag="gsum")
        nc.vector.tensor_reduce(out=gsum[:],
                                in_=pgv[:, :, 0:D].rearrange("p c f -> p f c"),
                                op=OP.add, axis=mybir.AxisListType.X)
        gv = pool.tile([P, 1], fp32, tag="gv")
        nc.sync.dma_start(out=gv[:], in_=ginv[t:t + 1, :].rearrange("o p -> p o"))
        nc.vector.tensor_scalar_mul(gsum[:], gsum[:], gv[:])
        psT = psp.tile([P, P], fp32, space="PSUM", tag="psT")
        nc.tensor.transpose(out=psT[:D, :], in_=gsum[:], identity=ident32[:])
        gT = pool.tile([D, P], fp32, tag="gT")
        nc.vector.tensor_copy(out=gT[:], in_=psT[:D, :])
        ps1 = psp.tile([P, 35], fp32, space="PSUM", tag="psmm")
        nc.tensor.matmul(out=ps1[:], lhsT=gT[:], rhs=w1_sb[:, :], start=True, stop=True)
        y1 = pool.tile([P, 35], fp32, tag="y1")
        nc.vector.tensor_tensor(out=y1[:], in0=ps1[:], in1=reps_sb[:, boff:boff + 35], op=OP.add)
        nc.scalar.activation(out=y1[:], in_=y1[:], func=AF.Relu)
        psT2 = psp.tile([P, P], fp32, space="PSUM", tag="psT")
        nc.tensor.transpose(out=psT2[:35, :], in_=y1[:], identity=ident32[:])
        y1T = pool.tile([35, P], fp32, tag="y1T")
        nc.vector.tensor_copy(out=y1T[:], in_=psT2[:35, :])
        ps2 = psp.tile([P, 17], fp32, space="PSUM", tag="psmm")
        nc.tensor.matmul(out=ps2[:], lhsT=y1T[:], rhs=w2_sb[:, :], start=True, stop=True)
        y2 = pool.tile([P, 17], fp32, tag="y2")
        nc.vector.tensor_tensor(out=y2[:], in0=ps2[:], in1=reps_sb[:, boff + 35:boff + 52], op=OP.add)
        nc.scalar.activation(out=y2[:], in_=y2[:], func=AF.Relu)
        psT3 = psp.tile([P, P], fp32, space="PSUM", tag="psT")
        nc.tensor.transpose(out=psT3[:17, :], in_=y2[:], identity=ident32[:])
        y2T = pool.tile([17, P], fp32, tag="y2T")
        nc.vector.tensor_copy(out=y2T[:], in_=psT3[:17, :])
        ps3 = psp.tile([P, 1], fp32, space="PSUM", tag="psmm")
        nc.tensor.matmul(out=ps3[:], lhsT=y2T[:], rhs=w3_sb[:, :], start=True, stop=True)
        y3 = pool.tile([P, 1], fp32, tag="y3")
        nc.vector.tensor_tensor(out=y3[:], in0=ps3[:], in1=reps_sb[:, boff + 52:boff + 53], op=OP.add)
        nc.sync.dma_start(out=out_g[t * P:(t + 1) * P, :], in_=y3[:])


def kernel(x, edge_index, edge_attr, batch, atom_emb, post_w, post_b,
           bn_gamma, bn_beta, mlp_w1, mlp_b1, mlp_w2, mlp_b2, mlp_w3, mlp_b3):
    cfg, arrays, asm = _prep(x, edge_index, edge_attr, batch, atom_emb)
    wd = _prep_weights(post_w, post_b, bn_gamma, bn_beta, mlp_w1, mlp_b1,
                       mlp_w2, mlp_b2, mlp_w3, mlp_b3)
    nc = _build(cfg)

    in_maps = []
    for c in range(NC):
        in_maps.append({
            "h0_own": arrays["h0_own"][c],
            "h0_full": arrays["h0_full"],
            "idx16": arrays["idx16"][c],
            "lane_m": arrays["lane_m"][c].reshape(4, P, -1),
            "eaflat": arrays["eaflat"][c],
            "statn16": np.repeat(arrays["statn"][c].transpose(2, 0, 1).reshape(P, -1),
                                 2, axis=1).astype(np.float16),
            "reps16": wd["reps"].astype(np.float16),
            "bigpad": arrays["bigpad"][c],
            "poolidx16": arrays["poolidx16"][c],
            "ginv": arrays["ginv"][c],
            "wch": wd["wch"],
            "reps": wd["reps"],
            "w1": wd["w1"],
            "w2": wd["w2"],
            "w3": wd["w3"],
        })
    import os
    trace = os.environ.get("KERNEL_TRACE", "0") == "1"
    res = run_bass_kernel_spmd(nc, in_maps, core_ids=list(range(NC)), trace=trace)
    kernel.last_exec_time_ns = res.exec_time_ns
    y = np.zeros((G, 1), np.float32)
    for c in range(NC):
        og = res.results[c]["out_g"]
        cg = asm["core_graphs"][c]
        y[cg] = og[:len(cg)]
    return y

